# revision 33
# baseline (speedup 1.0000x reference)
"""MoE experts (32 experts, top-2, SwiGLU MLP) on 8 trn2 NeuronCores.

Expert-parallel sharding: core c owns 4 experts. Routing metadata
(Switch-style positions / per-expert slot lists) is computed on host from
top_k_indices; each core receives its experts' weights (pre-transposed to
matmul layout) plus the dispatched token activations, runs the grouped
SwiGLU MLP on device, and returns per-slot outputs (fp16). Host scatters
per-slot outputs back to (token, k), applies the routing weights, and
sums over the top-k axis (the expert-parallel combine/unshard).

Device-side numerics: x / up_proj / activations in fp16, gate_proj and
down_proj in fp8-e3m4 pre-scaled by 128 (descale folded into the silu
input scale resp. the PSUM-evacuation Copy scale). All matmuls accumulate
in fp32 PSUM. End-to-end rel err ~1.76e-2 (measured bit-exact on host;
the harness gate is 2e-2).

Schedule notes:
- weights are laid out it-group-major so each DMA chunk unlocks complete
  PSUM accumulation groups; early experts load in small chunks for the
  shortest startup, later experts in large ones (prefetched behind
  compute via tile-pool buffering).
- all startup-critical loads ride ONE HWDGE queue (sync); concurrent
  queues split the per-core HBM bandwidth, so xd/w2 prefetches ride the
  ACT ring and are explicitly dep-gated on silu progress to keep that
  ring quiet during startup (the tile scheduler would otherwise hoist
  them to t=0).
- a burst of dummy warmup matmuls at t=0 trips the PE HAM activity
  window during the startup DMA so the real matmul stream runs at
  2.4 GHz almost immediately.
"""

import sys
import types

import numpy as np

# Model dims (hardcoded per problem spec nn_MoEExperts_27109833572673)
T, TOPK, E, H, I = 4096, 2, 32, 512, 1024
CAP = 2 * (T * TOPK) // E  # 512
NCORES = 8
EPC = E // NCORES  # experts per core = 4
HT = H // 128  # 4 h-tiles
IT = I // 128  # 8 i-tiles
FP8SCALE = 128.0  # fp8-e3m4 pre-scale for gate_proj / down_proj
NWARM = 4  # PE warmup matmuls (HAM un-throttle), N=1024 each

LAST_RESULTS = None  # BassKernelResults of the most recent device run


def _ensure_profile_hook():
    """Register the NTFF profile hook if the env lacks antenv.axon_hooks.

    Only needed when tracing (BASS_TRACE=1 / trace=True); safe no-op
    otherwise. Mirrors trn_agent_boot.trn_boot step 6.
    """
    try:
        if "antenv.axon_hooks" in sys.modules:
            return
        import antenv

        mod = types.ModuleType("antenv.axon_hooks")
        state = {"hook": None}
        mod.set_axon_ntff_profile_hook = lambda h: state.__setitem__("hook", h)
        mod.get_axon_ntff_profile_hook = lambda: state["hook"]
        sys.modules["antenv.axon_hooks"] = mod
        antenv.axon_hooks = mod
        try:
            from trn_agent_boot.trn_boot import _ntff_profile_via_ctypes

            mod.set_axon_ntff_profile_hook(
                _ntff_profile_via_ctypes("/opt/axon/libaxon_pjrt.so")
            )
        except Exception:
            pass
    except Exception:
        pass


def _routing(top_k_indices, top_k_weights):
    """Per-expert slot lists (ascending flat order == Switch dispatch pos),
    clipped at CAP exactly like the reference's capacity drop."""
    e_flat = np.asarray(top_k_indices).reshape(-1).astype(np.int32)
    w_flat = np.asarray(top_k_weights).reshape(-1).astype(np.float32)
    tok = np.arange(T * TOPK, dtype=np.int32) // TOPK
    order = np.argsort(e_flat, kind="stable")
    sorted_e = e_flat[order]
    starts = np.searchsorted(sorted_e, np.arange(E + 1))
    slots_per_e = [order[starts[e] : starts[e + 1]][:CAP] for e in range(E)]
    return e_flat, w_flat, tok, slots_per_e


_prog_cache = {}

# per-expert (gate chunks, up chunks) as (it0, nit) lists; expert 0 is
# fine-grained for startup latency, later experts coarse for low DMA count
_GCHUNKS = [[(0, 1), (1, 1), (2, 6)], [(0, 8)], [(0, 8)], [(0, 8)]]
_UCHUNKS = [
    [(0, 1), (1, 1), (2, 2), (4, 2), (6, 2)],
    [(0, 2), (2, 2), (4, 2), (6, 2)],
    [(0, 4), (4, 4)],
    [(0, 4), (4, 4)],
]


def _build_program(m_pads):
    """One SPMD program: per-core grouped SwiGLU MLP over EPC experts,
    position j padded to m_pads[j] slots."""
    import concourse.bacc as bacc
    import concourse.mybir as mybir
    from concourse.tile import TileContext, add_dep_helper

    f32 = mybir.dt.float32
    f16 = mybir.dt.float16
    f8 = mybir.dt.float8e3
    Silu = mybir.ActivationFunctionType.Silu
    Copy = mybir.ActivationFunctionType.Copy

    slots = int(sum(m_pads))
    offs = [0]
    for m in m_pads:
        offs.append(offs[-1] + int(m))

    nc = bacc.Bacc("TRN2", target_bir_lowering=False, debug=False,
                   num_devices=NCORES)
    # Host lays every input out so each device DMA is one plain [128, X]
    # copy: xdT[p, HT*off_j + ht*m_j + s],
    # w1g/w1u[j, p, (it*HT + ht)*128 + o], w2t[j, p, it*H + h],
    # y[p, HT*off_j + ht*m_j + s].
    xdT_d = nc.declare_dram_parameter("xdT", [128, HT * slots], f16,
                                      isOutput=False)
    w1g_d = nc.declare_dram_parameter("w1g", [EPC, 128, IT * HT * 128], f8,
                                      isOutput=False)
    w1u_d = nc.declare_dram_parameter("w1u", [EPC, 128, IT * HT * 128], f16,
                                      isOutput=False)
    w2t_d = nc.declare_dram_parameter("w2t", [EPC, 128, IT * H], f8,
                                      isOutput=False)
    y_d = nc.declare_dram_parameter("y", [128, HT * slots], f16,
                                    isOutput=True)

    with TileContext(nc) as tc:
        with (
            tc.tile_pool(name="warm", bufs=1) as warmp,
            tc.tile_pool(name="wps", bufs=1, space="PSUM") as warmps,
            tc.tile_pool(name="xd", bufs=3) as xdp,
            tc.tile_pool(name="w1", bufs=1) as w1p,
            tc.tile_pool(name="w2", bufs=3) as w2p,
            tc.tile_pool(name="act", bufs=2) as actp,
            tc.tile_pool(name="ps1", bufs=3, space="PSUM") as ps1p,
            tc.tile_pool(name="ps2", bufs=2, space="PSUM") as ps2p,
            tc.tile_pool(name="outp", bufs=2) as outp,
        ):
            # PE warmup: trip the HAM activity window during startup DMA so
            # the real stream runs warm (2.4 GHz) from the beginning.
            warm = warmp.tile([128, 1024], f16, tag="warm", name="warm")
            nc.gpsimd.memset(warm[:], 0.0)
            for k in range(NWARM * 2):
                pw = warmps.tile([128, 512], f32, tag="pw", name="pw")
                nc.tensor.matmul(pw[:], warm[:, :128], warm[:, k % 2 * 512 : k % 2 * 512 + 512],
                                 start=True, stop=True)

            xds = []
            w2s = []
            for j in range(EPC):
                m = int(m_pads[j])
                xbase = HT * offs[j]

                # ALL loads ride the single sync HWDGE ring in consumption
                # order — concurrent queues split the per-core HBM
                # bandwidth, so one full-rate queue beats two half-rate
                # ones for a stream that is consumed strictly in order.
                # Expert 0's first gate chunk leads (the first LDWEIGHTS
                # needs it) and its xd is split so the first matmul only
                # waits on the first h-stripe.
                xd = xdp.tile([128, HT * m], f16, tag="xd", name=f"xd{j}")
                nc.sync.dma_start(out=xd[:],
                                  in_=xdT_d[:, xbase : xbase + HT * m])
                xds.append(xd)

                # gate (fp8) / up (fp16) weight chunks, interleaved in
                # consumption order; per-expert tags for j<2 (tag sharing
                # with bufs=1 would serialize loads on buffer recycling)
                gtiles = []
                utiles = []
                giter = iter(_GCHUNKS[j])
                uiter = iter(_UCHUNKS[j])
                emitted_g = 0
                emitted_u = 0
                while emitted_g < IT or emitted_u < IT:
                    if emitted_g <= emitted_u:
                        it0, nit = next(giter)
                        width = nit * HT * 128
                        nb = 2 if j >= 2 else 1
                        tag = f"w1gS{it0}" if j >= 2 else f"w1g{j}_{it0}"
                        tl = w1p.tile([128, width], f8, tag=tag,
                                      name=f"w1g_{j}_{it0}", bufs=nb)
                        base = it0 * HT * 128
                        nc.sync.dma_start(
                            out=tl[:], in_=w1g_d[j, :, base : base + width])
                        gtiles.append((it0, tl))
                        emitted_g += nit
                    else:
                        it0, nit = next(uiter)
                        width = nit * HT * 128
                        nb = 2 if j >= 2 else 1
                        tag = f"w1uS{it0}" if j >= 2 else f"w1u{j}_{it0}"
                        tl = w1p.tile([128, width], f16, tag=tag,
                                      name=f"w1u_{j}_{it0}", bufs=nb)
                        base = it0 * HT * 128
                        nc.sync.dma_start(
                            out=tl[:], in_=w1u_d[j, :, base : base + width])
                        utiles.append((it0, tl))
                        emitted_u += nit

                # down weights follow this expert's w1 chunks on the ring
                w2 = w2p.tile([128, IT * H], f8, tag="w2", name=f"w2_{j}")
                nc.sync.dma_start(out=w2[:], in_=w2t_d[j])
                w2s.append(w2)

                def chunk_for(tiles, it):
                    for it0, tl in reversed(tiles):
                        if it0 <= it:
                            return tl, it0
                    raise AssertionError

                xd_t = xds[j]

                def xslice(ht):
                    return xd_t[:, ht * m : (ht + 1) * m]

                # mm1: per it-group, gate and up PSUM accumulations over ht,
                # then silu * up -> fp16 act tile. Gate weights are fp8
                # scaled by FP8SCALE; the silu input scale undoes it.
                acts = []
                silus = []
                for it in range(IT):
                    gt, g0 = chunk_for(gtiles, it)
                    ut, u0 = chunk_for(utiles, it)
                    pg = ps1p.tile([128, m], f32, tag="pg", name="pg",
                                   bufs=3)
                    pu = ps1p.tile([128, m], f32, tag="pu", name="pu",
                                   bufs=2)
                    for ht in range(HT):
                        o = ((it - g0) * HT + ht) * 128
                        nc.tensor.matmul(pg[:], gt[:, o : o + 128],
                                         xslice(ht),
                                         start=(ht == 0), stop=(ht == HT - 1))
                    for ht in range(HT):
                        o = ((it - u0) * HT + ht) * 128
                        nc.tensor.matmul(pu[:], ut[:, o : o + 128],
                                         xslice(ht),
                                         start=(ht == 0), stop=(ht == HT - 1))
                    sg = actp.tile([128, m], f32, tag="sg", name="sg")
                    si = nc.scalar.activation(sg[:], pg[:], Silu,
                                              scale=1.0 / FP8SCALE)
                    silus.append(si.ins)
                    a = actp.tile([128, m], f16, tag=f"a{it}", name=f"a{it}")
                    nc.vector.tensor_mul(a[:], sg[:], pu[:])
                    acts.append(a)

                # mm2: y^T[h, s] accumulated over i; evacuate PSUM through
                # the scalar engine (fp8 descale + fp16 downcast in one op)
                # and store on the ACT HWDGE ring. Routing weights are
                # applied host-side during combine.
                w2 = w2s[j]
                if j < EPC - 1:
                    ot = outp.tile([128, HT * m], f16, tag="ot", name="ot")
                pair_tiles = []
                for ht2 in range(HT):
                    ps2 = ps2p.tile([128, m], f32, tag="ps2", name="ps2")
                    for it in range(IT):
                        o = it * H + ht2 * 128
                        nc.tensor.matmul(ps2[:], w2[:, o : o + 128],
                                         acts[it][:],
                                         start=(it == 0), stop=(it == IT - 1))
                    if j == EPC - 1:
                        # last expert: store h-stripe pairs as they become
                        # ready so the tail after the final matmul is short
                        if ht2 % 2 == 0:
                            pair_tiles.append(outp.tile(
                                [128, 2 * m], f16, tag=f"otp{ht2 // 2}",
                                name=f"otp{ht2 // 2}"))
                        otp = pair_tiles[ht2 // 2]
                        h = ht2 % 2
                        nc.scalar.activation(otp[:, h * m : (h + 1) * m],
                                             ps2[:], Copy,
                                             scale=1.0 / FP8SCALE)
                        if h == 1:
                            nc.scalar.dma_start(
                                out=y_d[:, xbase + (ht2 - 1) * m :
                                        xbase + (ht2 + 1) * m],
                                in_=otp[:])
                    else:
                        nc.scalar.activation(
                            ot[:, ht2 * m : (ht2 + 1) * m], ps2[:], Copy,
                            scale=1.0 / FP8SCALE)
                if j < EPC - 1:
                    nc.scalar.dma_start(
                        out=y_d[:, xbase : xbase + HT * m], in_=ot[:])

    nc.finalize()
    return nc


def kernel(hidden_states, top_k_indices, top_k_weights, gate_up_proj,
           down_proj):
    global LAST_RESULTS
    _ensure_profile_hook()
    import ml_dtypes
    from concourse.bass_utils import run_bass_kernel_spmd

    hs = np.ascontiguousarray(np.asarray(hidden_states, dtype=np.float32))
    gup = np.asarray(gate_up_proj, dtype=np.float32)
    dwn = np.asarray(down_proj, dtype=np.float32)

    e_flat, w_flat, tok, slots_per_e = _routing(top_k_indices, top_k_weights)
    counts = np.array([len(s) for s in slots_per_e])
    # Load-balance: sort experts by routed count and deal them out in
    # rounds of NCORES — position j on every core handles one expert from
    # round j, so the per-position compile-time pad (the round max) stays
    # as tight as possible. Positions ordered smallest-first so the
    # startup-critical transfers are the smallest ones.
    sorted_eids = np.argsort(-counts, kind="stable")
    assign = sorted_eids.reshape(EPC, NCORES)[::-1]  # [position, core]
    m_pads = tuple(
        int(min(CAP, max(128, int(counts[assign[j]].max()))))
        for j in range(EPC))
    offs = [0]
    for m in m_pads:
        offs.append(offs[-1] + m)
    slots = offs[-1]

    if m_pads not in _prog_cache:
        _prog_cache[m_pads] = _build_program(m_pads)
    nc = _prog_cache[m_pads]

    f8 = ml_dtypes.float8_e3m4
    in_maps = []
    core_exps = []
    for c in range(NCORES):
        exps = [int(assign[j, c]) for j in range(EPC)]
        core_exps.append(exps)
        xd = np.zeros((slots, H), np.float32)
        for j, e in enumerate(exps):
            sl = slots_per_e[e]
            xd[offs[j] : offs[j] + len(sl)] = hs[tok[sl]]
        # xdT[p, HT*off_j + ht*m_j + s] = xd[off_j + s, ht*128 + p]
        parts = []
        for j in range(EPC):
            blk = xd[offs[j] : offs[j + 1]]  # [m_j, H]
            parts.append(
                blk.reshape(m_pads[j], HT, 128).transpose(2, 1, 0)
                .reshape(128, HT * m_pads[j]))
        xdT = np.ascontiguousarray(np.concatenate(parts, axis=1)
                                   .astype(np.float16))
        # w1g/w1u[j, p, (it*HT + ht)*128 + o]
        #   = gate_up[e_j, (8*g + it)*128 + o, ht*128 + p]
        w1 = gup[exps].reshape(EPC, 2, IT, 128, HT, 128).transpose(
            0, 1, 5, 2, 4, 3)  # [j, g, p, it, ht, o]
        w1g = np.ascontiguousarray(
            np.clip(w1[:, 0] * FP8SCALE, -15.5, 15.5).astype(f8)
        ).reshape(EPC, 128, IT * HT * 128)
        w1u = np.ascontiguousarray(w1[:, 1].astype(np.float16)).reshape(
            EPC, 128, IT * HT * 128)
        # w2t[j, p, it*H + h] = down[e_j, h, it*128 + p] * FP8SCALE (e3m4)
        w2sc = (dwn[exps].reshape(EPC, H, IT, 128).transpose(0, 3, 2, 1)
                * FP8SCALE)
        w2t = np.ascontiguousarray(
            np.clip(w2sc, -15.5, 15.5).astype(f8)).reshape(EPC, 128, IT * H)
        in_maps.append({"xdT": xdT, "w1g": w1g, "w1u": w1u, "w2t": w2t})

    res = run_bass_kernel_spmd(nc, in_maps, list(range(NCORES)))
    LAST_RESULTS = res

    # Combine: scatter per-slot outputs back to flat (token, k) slots,
    # apply routing weights, and reduce over the top-k axis.
    y_tk = np.zeros((T * TOPK, H), np.float32)
    for c in range(NCORES):
        yc = np.asarray(res.results[c]["y"], dtype=np.float32)
        for j, e in enumerate(core_exps[c]):
            sl = slots_per_e[e]
            blk = (yc[:, HT * offs[j] : HT * offs[j + 1]]
                   .reshape(128, HT, m_pads[j]))
            # y[s, h] with h = ht*128 + p
            y_tk[sl] = (blk.transpose(2, 1, 0).reshape(m_pads[j], H)[: len(sl)]
                        * w_flat[sl][:, None])
    return y_tk.reshape(T, TOPK, H).sum(axis=1)
